# revision 1
# baseline (speedup 1.0000x reference)
"""MinGRU layer Trainium2 kernel.

Reference semantics (B=8, T=16384, D=H=O=256):
    zs = sigmoid(xs @ Wz.T + bz);  hs = xs @ Wh.T + bh
    a = concat([1], 1-zs);  b = concat([0], zs*hs)         (T+1 positions)
    states = jax.lax.associative_scan(combine, (a, b))[1][:, 1:]
    out = states @ Wo.T + bo
with combine((a0,b0),(a1,b1)) = (a0*b0, b0*a1 + b1).

The combine is NOT associative, so the result is defined by jax's exact
odd/even recursion tree.  We replicate that tree exactly:
  - positions split into 8 aligned chunks of L=2048 (+1 trailing position);
  - per-chunk bottom-up reduce ("up-sweep", keeping all tree levels);
  - a tiny cross-chunk scan over the 8 chunk-top elements following the same
    recursion (chunk prefixes + chunk-end outputs);
  - per-chunk top-down "down-sweep" filling every position's scan value.

Sharding: batch b=8 across the 8 cores (one sequence per core); weights
replicated.  The host pre-transposes/casts x and the weights.

Engine layout: PE does the three matmuls (bf16); ACT does sigmoids, PSUM
evacuation and the f32->bf16 state cast; the whole scan tree runs on DVE
(GpSimd shares SBUF ports with DVE and would serialize against it).  Both
hidden halves are fused per op via [128, 2, n] APs.  Emission is software-
pipelined: phase-1 of chunk c+1 is emitted between chunk c's up-sweep and
down-sweep so its matmul/sigmoid/b-ops interleave into the engine queues
around the DVE tree chains, keeping PE/ACT busy while DVE walks the tree.
Per-core output is [O, T] without the output bias; the host transposes and
adds bo.
"""

from contextlib import ExitStack

import numpy as np
import ml_dtypes

import concourse.bacc as bacc
import concourse.tile as tile
from concourse import mybir
from concourse.bass_utils import run_bass_kernel_spmd

BF16 = ml_dtypes.bfloat16
F32 = mybir.dt.float32
BF = mybir.dt.bfloat16

B, T, D, H, O = 8, 16384, 256, 256, 256
L = 2048          # positions per chunk (power of 2)
NCHUNK = T // L   # 8 full chunks; position T (=16384) handled separately
SUB = 512         # matmul sub-chunk (one PSUM bank at f32)
LMAX = 11         # log2(L)

AluOp = mybir.AluOpType
ActFn = mybir.ActivationFunctionType


def _level_offsets():
    off = {1: 0}
    n = L // 2
    for lvl in range(1, LMAX):
        off[lvl + 1] = off[lvl] + n
        n //= 2
    return off, off[LMAX] + 1


LVL_OFF, LVL_TOTAL = _level_offsets()  # total = 2047


def build_nc():
    nc = bacc.Bacc()

    xt = nc.dram_tensor("xt", [D, T], BF, kind="ExternalInput")
    wzt = nc.dram_tensor("wzt", [D, H], BF, kind="ExternalInput")
    wht = nc.dram_tensor("wht", [D, H], BF, kind="ExternalInput")
    wot = nc.dram_tensor("wot", [H, O], BF, kind="ExternalInput")
    bzp = nc.dram_tensor("bzp", [H, 1], F32, kind="ExternalInput")   # +bz
    bzn = nc.dram_tensor("bzn", [H, 1], F32, kind="ExternalInput")   # -bz
    bhb = nc.dram_tensor("bhb", [H, 1], F32, kind="ExternalInput")
    out = nc.dram_tensor("out", [O, T], F32, kind="ExternalOutput")

    with tile.TileContext(nc) as tc, ExitStack() as ctx:
        singles = ctx.enter_context(tc.tile_pool(name="singles", bufs=1))
        ab_pool = ctx.enter_context(tc.tile_pool(name="ab", bufs=2))
        lvl_pool = ctx.enter_context(tc.tile_pool(name="lvl", bufs=1))
        dbuf_pool = ctx.enter_context(tc.tile_pool(name="dbuf", bufs=2))
        st_pool = ctx.enter_context(tc.tile_pool(name="st", bufs=2))
        tmp_pool = ctx.enter_context(tc.tile_pool(name="tmp", bufs=3))
        z_pool = ctx.enter_context(tc.tile_pool(name="zp", bufs=3))
        x_pool = ctx.enter_context(tc.tile_pool(name="xp", bufs=2))
        osb_pool = ctx.enter_context(tc.tile_pool(name="osb", bufs=2))
        psum_y = ctx.enter_context(tc.tile_pool(name="psy", bufs=3, space="PSUM"))
        psum_o = ctx.enter_context(tc.tile_pool(name="pso", bufs=1, space="PSUM"))

        # ---- constants ----
        wz_sb, wh_sb, wo_sb = [], [], []
        for k in range(2):
            wzk = singles.tile([128, H], BF, name=f"wzk{k}")
            nc.sync.dma_start(out=wzk, in_=wzt[k * 128:(k + 1) * 128, :])
            wz_sb.append(wzk)
            whk = singles.tile([128, H], BF, name=f"whk{k}")
            nc.sync.dma_start(out=whk, in_=wht[k * 128:(k + 1) * 128, :])
            wh_sb.append(whk)
            wok = singles.tile([128, O], BF, name=f"wok{k}")
            nc.sync.dma_start(out=wok, in_=wot[k * 128:(k + 1) * 128, :])
            wo_sb.append(wok)
        bzp_sb, bzn_sb, bh_sb = [], [], []
        for h in range(2):
            pz = singles.tile([128, 1], F32, name=f"bzp{h}")
            nc.sync.dma_start(out=pz, in_=bzp[h * 128:(h + 1) * 128, :])
            bzp_sb.append(pz)
            nz = singles.tile([128, 1], F32, name=f"bzn{h}")
            nc.sync.dma_start(out=nz, in_=bzn[h * 128:(h + 1) * 128, :])
            bzn_sb.append(nz)
            hb = singles.tile([128, 1], F32, name=f"bh{h}")
            nc.sync.dma_start(out=hb, in_=bhb[h * 128:(h + 1) * 128, :])
            bh_sb.append(hb)

        # top-level bookkeeping, both halves fused: [128, 2, n]
        tops_A = singles.tile([128, 2, 8], F32, name="topsA")
        tops_B = singles.tile([128, 2, 8], F32, name="topsB")
        # spine: 0-3 sB12_0..3, 4 sA12_1, 5 sA12_2, 6 sA12_3,
        #        7 sB13_0, 8 sB13_1, 9 sA13_1, 10 sB14
        spine = singles.tile([128, 2, 12], F32, name="spine")
        otb = singles.tile([128, 2, 8], F32, name="otb")

        def top_combine(dstB, lB, rA, rB):
            """combine on [128,2,1] APs (DVE): dstB = lB*rA + rB."""
            t = tmp_pool.tile([128, 2, 1], F32, name="ttop", tag="ttop")
            nc.vector.tensor_tensor(t, lB, rA, op=AluOp.mult)
            nc.vector.tensor_tensor(dstB, t, rB, op=AluOp.add)

        abufs = {}

        def emit_phase1(c):
            """DMA/matmul/sigmoid/b for chunk c into fresh a/b tiles."""
            a_buf = ab_pool.tile([128, 2, L], F32, name="a_buf", tag="a")
            b_buf = ab_pool.tile([128, 2, L], F32, name="b_buf", tag="b")
            abufs[c] = (a_buf, b_buf)
            if c == 0:
                nc.vector.memset(a_buf[:, :, 0:1], 1.0)
                nc.vector.memset(b_buf[:, :, 0:1], 0.0)
                subs = [(s * SUB, SUB if s < 3 else SUB - 1, s * SUB + 1)
                        for s in range(4)]
            else:
                base = c * L - 1
                subs = [(base + s * SUB, SUB, s * SUB) for s in range(4)]
            for x0, ncols, acol in subs:
                xk = x_pool.tile([128, 2, SUB], BF, name="xk", tag="xk")
                nc.sync.dma_start(
                    out=xk[:, :, :ncols],
                    in_=xt[:, x0:x0 + ncols].rearrange("(k p) n -> p k n", p=128))
                for h in range(2):
                    yz = psum_y.tile([128, SUB], F32, name="yz", tag=f"y{h}")
                    for k in range(2):
                        nc.tensor.matmul(yz[:, :ncols],
                                         wz_sb[k][:, h * 128:(h + 1) * 128],
                                         xk[:, k, :ncols],
                                         start=(k == 0), stop=(k == 1))
                    zt = z_pool.tile([128, SUB], F32, name="zt", tag=f"zt{h}")
                    nc.scalar.activation(zt[:, :ncols], yz[:, :ncols],
                                         ActFn.Sigmoid,
                                         bias=bzp_sb[h][:, 0:1], scale=1.0)
                    nc.scalar.activation(a_buf[:, h, acol:acol + ncols],
                                         yz[:, :ncols], ActFn.Sigmoid,
                                         bias=bzn_sb[h][:, 0:1], scale=-1.0)
                    yh = psum_y.tile([128, SUB], F32, name="yh", tag=f"y{h}")
                    for k in range(2):
                        nc.tensor.matmul(yh[:, :ncols],
                                         wh_sb[k][:, h * 128:(h + 1) * 128],
                                         xk[:, k, :ncols],
                                         start=(k == 0), stop=(k == 1))
                    nc.vector.scalar_tensor_tensor(
                        b_buf[:, h, acol:acol + ncols], yh[:, :ncols],
                        bh_sb[h][:, 0:1], zt[:, :ncols],
                        op0=AluOp.add, op1=AluOp.mult)

        def emit_up_top(c):
            a_buf, b_buf = abufs[c]
            # ---- phase 2: up-sweep (DVE) ----
            Aup = lvl_pool.tile([128, 2, LVL_TOTAL], F32, name="Aup", tag="Au")
            Bup = lvl_pool.tile([128, 2, LVL_TOTAL], F32, name="Bup", tag="Bu")
            for lvl in range(LMAX):
                n = L >> lvl
                m = n // 2
                if lvl == 0:
                    sA, sB = a_buf, b_buf
                else:
                    o = LVL_OFF[lvl]
                    sA = Aup[:, :, o:o + n]
                    sB = Bup[:, :, o:o + n]
                o2 = LVL_OFF[lvl + 1]
                dA = Aup[:, :, o2:o2 + m]
                dB = Bup[:, :, o2:o2 + m]
                A_ev, A_od = sA[:, :, 0:n:2], sA[:, :, 1:n:2]
                B_ev, B_od = sB[:, :, 0:n:2], sB[:, :, 1:n:2]
                nc.vector.tensor_tensor(dA, A_ev, B_ev, op=AluOp.mult)
                tu = tmp_pool.tile([128, 2, L // 2], F32, name="tu", tag="tmp")
                nc.vector.tensor_tensor(tu[:, :, :m], B_ev, A_od, op=AluOp.mult)
                nc.vector.tensor_tensor(dB, tu[:, :, :m], B_od, op=AluOp.add)

            # ---- phase 3: top-level bookkeeping (DVE) ----
            o11 = LVL_OFF[LMAX]
            EA = tops_A[:, :, c:c + 1]
            EB = tops_B[:, :, c:c + 1]
            nc.vector.tensor_copy(EA, Aup[:, :, o11:o11 + 1])
            nc.vector.tensor_copy(EB, Bup[:, :, o11:o11 + 1])
            sp = spine
            cc = lambda i: (tops_A[:, :, i:i + 1], tops_B[:, :, i:i + 1])
            if c == 0:
                nc.vector.tensor_copy(otb[:, :, 0:1], EB)
            elif c == 1:
                top_combine(sp[:, :, 0:1], cc(0)[1], *cc(1))
                nc.vector.tensor_copy(otb[:, :, 1:2], sp[:, :, 0:1])
            elif c == 2:
                top_combine(otb[:, :, 2:3], otb[:, :, 1:2], EA, EB)
            elif c == 3:
                top_combine(sp[:, :, 1:2], cc(2)[1], *cc(3))
                nc.vector.tensor_tensor(sp[:, :, 4:5], cc(2)[0], cc(2)[1],
                                        op=AluOp.mult)          # sA12_1
                top_combine(sp[:, :, 7:8], sp[:, :, 0:1],
                            sp[:, :, 4:5], sp[:, :, 1:2])       # sB13_0
                nc.vector.tensor_copy(otb[:, :, 3:4], sp[:, :, 7:8])
            elif c == 4:
                top_combine(otb[:, :, 4:5], otb[:, :, 3:4], EA, EB)
            elif c == 5:
                top_combine(sp[:, :, 2:3], cc(4)[1], *cc(5))    # sB12_2
                nc.vector.tensor_tensor(sp[:, :, 5:6], cc(4)[0], cc(4)[1],
                                        op=AluOp.mult)          # sA12_2
                top_combine(otb[:, :, 5:6], otb[:, :, 3:4],
                            sp[:, :, 5:6], sp[:, :, 2:3])
            elif c == 6:
                top_combine(otb[:, :, 6:7], otb[:, :, 5:6], EA, EB)
            elif c == 7:
                top_combine(sp[:, :, 3:4], cc(6)[1], *cc(7))    # sB12_3
                nc.vector.tensor_tensor(sp[:, :, 6:7], cc(6)[0], cc(6)[1],
                                        op=AluOp.mult)          # sA12_3
                top_combine(sp[:, :, 8:9], sp[:, :, 2:3],
                            sp[:, :, 6:7], sp[:, :, 3:4])       # sB13_1
                nc.vector.tensor_tensor(sp[:, :, 9:10], sp[:, :, 5:6],
                                        sp[:, :, 2:3], op=AluOp.mult)  # sA13_1
                top_combine(sp[:, :, 10:11], sp[:, :, 7:8],
                            sp[:, :, 9:10], sp[:, :, 8:9])      # sB14
                nc.vector.tensor_copy(otb[:, :, 7:8], sp[:, :, 10:11])

            return Aup, Bup

        def emit_down(c, Aup, Bup):
            a_buf, b_buf = abufs.pop(c)
            # ---- phase 4: down-sweep (DVE) ----
            dbuf = dbuf_pool.tile([128, 2, L + 1], F32, name="dbuf", tag="d")
            if c == 0:
                nc.vector.memset(dbuf[:, :, 0:1], 0.0)
            else:
                nc.vector.tensor_copy(dbuf[:, :, 0:1], otb[:, :, c - 1:c])
            nc.vector.tensor_copy(dbuf[:, :, L:L + 1], otb[:, :, c:c + 1])
            for lvl in range(LMAX - 1, -1, -1):
                n = L >> lvl
                cnt = n // 2
                step = 1 << (lvl + 1)
                if lvl == 0:
                    A_src, B_src = a_buf, b_buf
                else:
                    o = LVL_OFF[lvl]
                    A_src = Aup[:, :, o:o + n]
                    B_src = Bup[:, :, o:o + n]
                A_ev = A_src[:, :, 0:n:2]
                B_ev = B_src[:, :, 0:n:2]
                Lh = dbuf[:, :, 0:L:step]
                Wt = dbuf[:, :, (1 << lvl):L:step]
                td = tmp_pool.tile([128, 2, L // 2], F32, name="td", tag="tmp")
                nc.vector.tensor_tensor(td[:, :, :cnt], Lh, A_ev, op=AluOp.mult)
                nc.vector.tensor_tensor(Wt, td[:, :, :cnt], B_ev, op=AluOp.add)
            return dbuf

        def emit_out(c, dbuf):
            # ---- phase 5: cast + output matmul + store ----
            # states live in dbuf cols [1, 2049); chunk 0's col 1 is the dummy
            # position-0 value, skipped at DMA time.
            obase = c * L - 1
            st = st_pool.tile([128, 2, L], BF, name="st", tag="st")
            for s in range(4):
                col0 = s * SUB
                nc.scalar.copy(st[:, :, col0:col0 + SUB],
                               dbuf[:, :, 1 + col0:1 + col0 + SUB])
                po = psum_o.tile([128, 2, SUB], F32, name="po", tag="po")
                for oh in range(2):
                    for k in range(2):
                        nc.tensor.matmul(po[:, oh, :],
                                         wo_sb[k][:, oh * 128:(oh + 1) * 128],
                                         st[:, k, col0:col0 + SUB],
                                         start=(k == 0), stop=(k == 1))
                osb = osb_pool.tile([128, 2, SUB], F32, name="osb", tag="osb")
                nc.scalar.copy(osb, po)
                skip = 1 if (c == 0 and s == 0) else 0
                dst = out[:, obase + col0 + skip:obase + col0 + SUB]
                nc.sync.dma_start(
                    out=dst.rearrange("(two p) n -> p two n", p=128),
                    in_=osb[:, :, skip:])

        # ---- software-pipelined emission: phase1(c+1) is emitted between
        # chunk c's up-sweep and down-sweep so its matmul/sigmoid/b work
        # overlaps the rest of chunk c's DVE chain without head-blocking ----
        emit_phase1(0)
        for c in range(NCHUNK):
            Aup_c, Bup_c = emit_up_top(c)
            if c + 1 < NCHUNK:
                emit_phase1(c + 1)
            dbuf_c = emit_down(c, Aup_c, Bup_c)
            emit_out(c, dbuf_c)
            last_dbuf = dbuf_c

        # ---- final position T: out[p] = out[p-1]*a + b ----
        xl = singles.tile([128, 2, 1], BF, name="xl")
        nc.sync.dma_start(out=xl,
                          in_=xt[:, T - 1:T].rearrange("(k p) n -> p k n", p=128))
        al = singles.tile([128, 2, 1], F32, name="al")
        bl = singles.tile([128, 2, 1], F32, name="bl")
        for h in range(2):
            yzl = psum_y.tile([128, SUB], F32, name="yzl", tag=f"y{h}")[:, 0:1]
            for k in range(2):
                nc.tensor.matmul(yzl, wz_sb[k][:, h * 128:(h + 1) * 128],
                                 xl[:, k, :], start=(k == 0), stop=(k == 1))
            zl = singles.tile([128, 1], F32, name=f"zl{h}")
            nc.scalar.activation(zl, yzl, ActFn.Sigmoid,
                                 bias=bzp_sb[h][:, 0:1], scale=1.0)
            nc.scalar.activation(al[:, h, :], yzl, ActFn.Sigmoid,
                                 bias=bzn_sb[h][:, 0:1], scale=-1.0)
            yhl = psum_y.tile([128, SUB], F32, name="yhl", tag=f"y{h}")[:, 0:1]
            for k in range(2):
                nc.tensor.matmul(yhl, wh_sb[k][:, h * 128:(h + 1) * 128],
                                 xl[:, k, :], start=(k == 0), stop=(k == 1))
            nc.vector.scalar_tensor_tensor(bl[:, h, :], yhl, bh_sb[h][:, 0:1],
                                           zl, op0=AluOp.add, op1=AluOp.mult)
        dl = singles.tile([128, 2, 1], F32, name="dl")
        sl = singles.tile([128, 2, 1], BF, name="sl")
        nc.vector.tensor_tensor(dl, last_dbuf[:, :, L:L + 1], al, op=AluOp.mult)
        nc.vector.tensor_tensor(dl, dl, bl, op=AluOp.add)
        nc.scalar.copy(sl, dl)
        pol = psum_o.tile([128, 2, SUB], F32, name="pol", tag="po")[:, :, 0:1]
        for oh in range(2):
            for k in range(2):
                nc.tensor.matmul(pol[:, oh, :],
                                 wo_sb[k][:, oh * 128:(oh + 1) * 128],
                                 sl[:, k, :], start=(k == 0), stop=(k == 1))
        osl = singles.tile([128, 2, 1], F32, name="osl")
        nc.scalar.copy(osl, pol)
        nc.sync.dma_start(
            out=out[:, T - 1:T].rearrange("(two p) n -> p two n", p=128),
            in_=osl)

    nc.compile()
    return nc


_NC_CACHE = {}


def _get_nc():
    if "nc" not in _NC_CACHE:
        _NC_CACHE["nc"] = build_nc()
    return _NC_CACHE["nc"]


def _prepare_in_maps(xs, Wz, bz, Wh, bh, Wo, bo):
    xs = np.asarray(xs, np.float32)
    Wz = np.asarray(Wz, np.float32)
    bz = np.asarray(bz, np.float32)
    Wh = np.asarray(Wh, np.float32)
    bh = np.asarray(bh, np.float32)
    Wo = np.asarray(Wo, np.float32)

    wzt = np.ascontiguousarray(Wz.T).astype(BF16)
    wht = np.ascontiguousarray(Wh.T).astype(BF16)
    wot = np.ascontiguousarray(Wo.T).astype(BF16)
    bzp = np.ascontiguousarray(bz.reshape(H, 1))
    bzn = np.ascontiguousarray((-bz).reshape(H, 1))
    bhb = np.ascontiguousarray(bh.reshape(H, 1))

    in_maps = []
    for i in range(B):
        xti = np.ascontiguousarray(xs[i].T).astype(BF16)
        in_maps.append({
            "xt": xti, "wzt": wzt, "wht": wht, "wot": wot,
            "bzp": bzp, "bzn": bzn, "bhb": bhb,
        })
    return in_maps


def _assemble(res, bo):
    bo = np.asarray(bo, np.float32)
    return np.stack([np.asarray(res.results[i]["out"], np.float32).T + bo
                     for i in range(B)], axis=0)


def run_traced(xs, Wz, bz, Wh, bh, Wo, bo, trace=True):
    in_maps = _prepare_in_maps(xs, Wz, bz, Wh, bh, Wo, bo)
    res = run_bass_kernel_spmd(_get_nc(), in_maps, core_ids=list(range(B)),
                               trace=trace)
    return _assemble(res, bo), res


def kernel(xs, Wz, bz, Wh, bh, Wo, bo):
    in_maps = _prepare_in_maps(xs, Wz, bz, Wh, bh, Wo, bo)
    res = run_bass_kernel_spmd(_get_nc(), in_maps, core_ids=list(range(B)))
    return _assemble(res, bo)



# revision 6
# speedup vs baseline: 1.5750x; 1.5750x over previous
"""MinGRU layer Trainium2 kernel — v2: fp16 bit-reversed packed tree.

Reference semantics (B=8, T=16384, D=H=O=256):
    zs = sigmoid(xs @ Wz.T + bz);  hs = xs @ Wh.T + bh
    a = concat([1], 1-zs);  b = concat([0], zs*hs)         (T+1 positions)
    states = jax.lax.associative_scan(combine, (a, b))[1][:, 1:]
    out = states @ Wo.T + bo
with combine((a0,b0),(a1,b1)) = (a0*b0, b0*a1 + b1) — non-associative, so
the result is defined by jax's exact odd/even recursion tree, replicated
here: 8 aligned chunks of L=2048 (+1 trailing position), per-chunk up/down
sweep, cross-chunk spine over the 8 chunk tops.

v2 layout: within each chunk the time axis is stored in BIT-REVERSED order
(host permutes x columns; host un-permutes the output).  Every tree level's
even/odd split is then a contiguous halving, so all up-sweep ops are packed
stride-1.  The down-sweep writes level slabs deepest-first into one buffer
st = [carry | E^10 | E^9 | ... | E^0 | top]; the shifted scan vector
V^{l+1} needed at level l is then exactly the contiguous prefix st[0:m].
All tree tensors are fp16: DVE's 2x_1p fast mode doubles throughput on
2-byte packed operands, and fp16's 10-bit mantissa keeps the numerics
~4x tighter than the bf16 baseline (values are range-safe: |A|,|B| < 3).

Engine split: PE runs the three matmuls (fp16 in, f32 PSUM); ACT does the
z sigmoid and the (yh + bh) PSUM evacuation per H-half; DVE does b = z*h,
a = 1 - z (tensor_scalar 4x mode) and the whole packed tree; output DMAs
go straight from PSUM to HBM (f32, no evacuation op).  Emission is
software-pipelined: phase-1 of chunk c+1 is emitted between chunk c's
up-sweep and down-sweep.  Per-core output is [O, T+1] without the output
bias (col T = trailing position); the host maps bit-rev slots back to time
positions, transposes, and adds bo.
"""

from contextlib import ExitStack

import numpy as np

import concourse.bacc as bacc
import concourse.tile as tile
from concourse import mybir
from concourse.bass_utils import run_bass_kernel_spmd

F32 = mybir.dt.float32
F16 = mybir.dt.float16
NP16 = np.float16

B, T, D, H, O = 8, 16384, 256, 256, 256
L = 2048          # positions per chunk (power of 2)
NCHUNK = T // L   # 8 full chunks; position T (=16384) handled separately
SUB = 512         # matmul sub-chunk
LMAX = 11         # log2(L)

AluOp = mybir.AluOpType
ActFn = mybir.ActivationFunctionType


def _level_offsets():
    off = {1: 0}
    n = L // 2
    for lvl in range(1, LMAX):
        off[lvl + 1] = off[lvl] + n
        n //= 2
    return off, off[LMAX] + 1


LVL_OFF, LVL_TOTAL = _level_offsets()  # total = 2047


def _brev_table():
    br = np.zeros(L, np.int64)
    for r in range(L):
        x, v = r, 0
        for _ in range(LMAX):
            v = (v << 1) | (x & 1)
            x >>= 1
        br[r] = v
    return br


BRT = _brev_table()


def _pos_of_slot():
    """st slot s (1..L) -> chunk-local scan position."""
    pos = np.full(L + 1, -1, np.int64)
    for lvl in range(LMAX - 1, -1, -1):
        m = L >> (lvl + 1)
        bits = m.bit_length() - 1
        for r in range(m):
            x, k = r, 0
            for _ in range(bits):
                k = (k << 1) | (x & 1)
                x >>= 1
            pos[m + r] = (1 << lvl) * (2 * k + 1) - 1
    pos[L] = L - 1
    return pos


POS_OF_SLOT = _pos_of_slot()


def build_nc():
    nc = bacc.Bacc()

    xt = nc.dram_tensor("xt", [D, T + 1], F16, kind="ExternalInput")
    wzt = nc.dram_tensor("wzt", [D, H], F16, kind="ExternalInput")
    wht = nc.dram_tensor("wht", [D, H], F16, kind="ExternalInput")
    wot = nc.dram_tensor("wot", [H, O], F16, kind="ExternalInput")
    bzp = nc.dram_tensor("bzp", [H, 1], F32, kind="ExternalInput")   # +bz
    bzn = nc.dram_tensor("bzn", [H, 1], F32, kind="ExternalInput")   # -bz
    bhb = nc.dram_tensor("bhb", [H, 1], F32, kind="ExternalInput")
    out = nc.dram_tensor("out", [O, T + 1], F32, kind="ExternalOutput")

    with tile.TileContext(nc) as tc, ExitStack() as ctx:
        singles = ctx.enter_context(tc.tile_pool(name="singles", bufs=1))
        x_pool = ctx.enter_context(tc.tile_pool(name="xp", bufs=2))
        ab_pool = ctx.enter_context(tc.tile_pool(name="ab", bufs=2))
        lvl_pool = ctx.enter_context(tc.tile_pool(name="lvl", bufs=2))
        st_pool = ctx.enter_context(tc.tile_pool(name="st", bufs=2))
        tmp_pool = ctx.enter_context(tc.tile_pool(name="tmp", bufs=2))
        zh_pool = ctx.enter_context(tc.tile_pool(name="zh", bufs=2))
        osb_pool = ctx.enter_context(tc.tile_pool(name="osb", bufs=2))
        spt_pool = ctx.enter_context(tc.tile_pool(name="spt", bufs=2))
        psum = ctx.enter_context(tc.tile_pool(name="ps", bufs=4, space="PSUM"))

        # ---- constants ----
        wz_sb, wh_sb, wo_sb = [], [], []
        for k in range(2):
            wzk = singles.tile([128, H], F16, name=f"wzk{k}")
            nc.sync.dma_start(out=wzk, in_=wzt[k * 128:(k + 1) * 128, :])
            wz_sb.append(wzk)
            whk = singles.tile([128, H], F16, name=f"whk{k}")
            nc.sync.dma_start(out=whk, in_=wht[k * 128:(k + 1) * 128, :])
            wh_sb.append(whk)
            wok = singles.tile([128, O], F16, name=f"wok{k}")
            nc.sync.dma_start(out=wok, in_=wot[k * 128:(k + 1) * 128, :])
            wo_sb.append(wok)
        bzp_sb, bzn_sb, bh_sb = [], [], []
        for h in range(2):
            pz = singles.tile([128, 1], F32, name=f"bzp{h}")
            nc.sync.dma_start(out=pz, in_=bzp[h * 128:(h + 1) * 128, :])
            bzp_sb.append(pz)
            nz = singles.tile([128, 1], F32, name=f"bzn{h}")
            nc.sync.dma_start(out=nz, in_=bzn[h * 128:(h + 1) * 128, :])
            bzn_sb.append(nz)
            hb = singles.tile([128, 1], F32, name=f"bh{h}")
            nc.sync.dma_start(out=hb, in_=bhb[h * 128:(h + 1) * 128, :])
            bh_sb.append(hb)

        # top-level bookkeeping, both halves fused: [128, 2, n] (f32)
        tops_A = singles.tile([128, 2, 8], F32, name="topsA")
        tops_B = singles.tile([128, 2, 8], F32, name="topsB")
        # spine: 0-3 sB12_0..3, 4 sA12_1, 5 sA12_2, 6 sA12_3,
        #        7 sB13_0, 8 sB13_1, 9 sA13_1, 10 sB14
        spine = singles.tile([128, 2, 12], F32, name="spine")
        otb = singles.tile([128, 2, 8], F32, name="otb")

        def top_combine(dstB, lB, rA, rB):
            t = spt_pool.tile([128, 2, 1], F32, name="ttop", tag="ttop")
            nc.vector.tensor_tensor(t, lB, rA, op=AluOp.mult)
            nc.vector.tensor_tensor(dstB, t, rB, op=AluOp.add)

        abufs = {}

        def emit_phase1(c):
            """DMA/matmul/sigmoid/h-evac/b/a for chunk c (bit-rev cols)."""
            x_t = x_pool.tile([128, 2, L], F16, name="x_t", tag="x")
            nc.sync.dma_start(
                out=x_t,
                in_=xt[:, c * L:(c + 1) * L].rearrange("(k p) n -> p k n",
                                                       p=128))
            a_buf = ab_pool.tile([128, 2, L], F16, name="a_buf", tag="a")
            b_buf = ab_pool.tile([128, 2, L], F16, name="b_buf", tag="b")
            abufs[c] = (a_buf, b_buf)
            for s in range(4):
                col = s * SUB
                yz = psum.tile([128, 2, SUB], F32, name="yz", tag="ps")
                for hh in range(2):
                    for k in range(2):
                        nc.tensor.matmul(yz[:, hh, :],
                                         wz_sb[k][:, hh * 128:(hh + 1) * 128],
                                         x_t[:, k, col:col + SUB],
                                         start=(k == 0), stop=(k == 1))
                yh = psum.tile([128, 2, SUB], F32, name="yh", tag="ps")
                for hh in range(2):
                    for k in range(2):
                        nc.tensor.matmul(yh[:, hh, :],
                                         wh_sb[k][:, hh * 128:(hh + 1) * 128],
                                         x_t[:, k, col:col + SUB],
                                         start=(k == 0), stop=(k == 1))
                z_t = zh_pool.tile([128, 2, SUB], F16, name="z_t", tag="z")
                h_t = zh_pool.tile([128, 2, SUB], F16, name="h_t", tag="h")
                for hh in range(2):
                    nc.scalar.activation(z_t[:, hh, :], yz[:, hh, :],
                                         ActFn.Sigmoid,
                                         bias=bzp_sb[hh][:, 0:1], scale=1.0)
                    nc.scalar.activation(h_t[:, hh, :], yh[:, hh, :],
                                         ActFn.Identity,
                                         bias=bh_sb[hh][:, 0:1], scale=1.0)
                # b = z * h   (fp16 packed, DVE 2x)
                nc.vector.tensor_tensor(b_buf[:, :, col:col + SUB], z_t, h_t,
                                        op=AluOp.mult)
                # a = 1 - z = (z * -1) + 1   (fp16 packed, DVE 4x)
                nc.vector.tensor_scalar(a_buf[:, :, col:col + SUB], z_t,
                                        -1.0, 1.0, op0=AluOp.mult,
                                        op1=AluOp.add)
            if c == 0:
                # slot 0 = bit-rev of position 0 = the prepended (1, 0)
                nc.vector.memset(a_buf[:, :, 0:1], 1.0)
                nc.vector.memset(b_buf[:, :, 0:1], 0.0)

        def emit_up_top(c):
            a_buf, b_buf = abufs[c]
            # ---- up-sweep: all contiguous halves (fp16 packed) ----
            Aup = lvl_pool.tile([128, 2, LVL_TOTAL], F16, name="Aup", tag="A")
            Bup = lvl_pool.tile([128, 2, LVL_TOTAL], F16, name="Bup", tag="B")
            for lvl in range(LMAX):
                n = L >> lvl
                h2 = n >> 1
                if lvl == 0:
                    sA, sB = a_buf, b_buf
                    o = 0
                else:
                    o = LVL_OFF[lvl]
                    sA = Aup[:, :, o:o + n]
                    sB = Bup[:, :, o:o + n]
                o2 = LVL_OFF[lvl + 1]
                A_lo, A_hi = sA[:, :, 0:h2], sA[:, :, h2:n]
                B_lo, B_hi = sB[:, :, 0:h2], sB[:, :, h2:n]
                nc.vector.tensor_tensor(Aup[:, :, o2:o2 + h2], A_lo, B_lo,
                                        op=AluOp.mult)
                q = tmp_pool.tile([128, 2, L // 2], F16, name="qu", tag="qu")
                nc.vector.tensor_tensor(q[:, :, :h2], B_lo, A_hi,
                                        op=AluOp.mult)
                nc.vector.tensor_tensor(Bup[:, :, o2:o2 + h2], q[:, :, :h2],
                                        B_hi, op=AluOp.add)

            # ---- cross-chunk spine (f32) ----
            o11 = LVL_OFF[LMAX]
            EA = tops_A[:, :, c:c + 1]
            EB = tops_B[:, :, c:c + 1]
            nc.vector.tensor_copy(EA, Aup[:, :, o11:o11 + 1])
            nc.vector.tensor_copy(EB, Bup[:, :, o11:o11 + 1])
            sp = spine
            cc = lambda i: (tops_A[:, :, i:i + 1], tops_B[:, :, i:i + 1])
            if c == 0:
                nc.vector.tensor_copy(otb[:, :, 0:1], EB)
            elif c == 1:
                top_combine(sp[:, :, 0:1], cc(0)[1], *cc(1))
                nc.vector.tensor_copy(otb[:, :, 1:2], sp[:, :, 0:1])
            elif c == 2:
                top_combine(otb[:, :, 2:3], otb[:, :, 1:2], EA, EB)
            elif c == 3:
                top_combine(sp[:, :, 1:2], cc(2)[1], *cc(3))
                nc.vector.tensor_tensor(sp[:, :, 4:5], cc(2)[0], cc(2)[1],
                                        op=AluOp.mult)          # sA12_1
                top_combine(sp[:, :, 7:8], sp[:, :, 0:1],
                            sp[:, :, 4:5], sp[:, :, 1:2])       # sB13_0
                nc.vector.tensor_copy(otb[:, :, 3:4], sp[:, :, 7:8])
            elif c == 4:
                top_combine(otb[:, :, 4:5], otb[:, :, 3:4], EA, EB)
            elif c == 5:
                top_combine(sp[:, :, 2:3], cc(4)[1], *cc(5))    # sB12_2
                nc.vector.tensor_tensor(sp[:, :, 5:6], cc(4)[0], cc(4)[1],
                                        op=AluOp.mult)          # sA12_2
                top_combine(otb[:, :, 5:6], otb[:, :, 3:4],
                            sp[:, :, 5:6], sp[:, :, 2:3])
            elif c == 6:
                top_combine(otb[:, :, 6:7], otb[:, :, 5:6], EA, EB)
            elif c == 7:
                top_combine(sp[:, :, 3:4], cc(6)[1], *cc(7))    # sB12_3
                nc.vector.tensor_tensor(sp[:, :, 6:7], cc(6)[0], cc(6)[1],
                                        op=AluOp.mult)          # sA12_3
                top_combine(sp[:, :, 8:9], sp[:, :, 2:3],
                            sp[:, :, 6:7], sp[:, :, 3:4])       # sB13_1
                nc.vector.tensor_tensor(sp[:, :, 9:10], sp[:, :, 5:6],
                                        sp[:, :, 2:3], op=AluOp.mult)  # sA13_1
                top_combine(sp[:, :, 10:11], sp[:, :, 7:8],
                            sp[:, :, 9:10], sp[:, :, 8:9])      # sB14
                nc.vector.tensor_copy(otb[:, :, 7:8], sp[:, :, 10:11])

            return Aup, Bup

        def emit_down(c, Aup, Bup):
            a_buf, b_buf = abufs.pop(c)
            # ---- down-sweep: st = [carry | E^10 | ... | E^0 | top] ----
            st = st_pool.tile([128, 2, L + 1], F16, name="st", tag="st")
            if c == 0:
                nc.vector.memset(st[:, :, 0:1], 0.0)
            else:
                nc.vector.tensor_copy(st[:, :, 0:1], otb[:, :, c - 1:c])
            nc.vector.tensor_copy(st[:, :, L:L + 1], otb[:, :, c:c + 1])
            for lvl in range(LMAX - 1, -1, -1):
                m = L >> (lvl + 1)
                if lvl == 0:
                    A_lo = a_buf[:, :, 0:m]
                    B_lo = b_buf[:, :, 0:m]
                else:
                    o = LVL_OFF[lvl]
                    A_lo = Aup[:, :, o:o + m]
                    B_lo = Bup[:, :, o:o + m]
                td = tmp_pool.tile([128, 2, L // 2], F16, name="td", tag="td")
                nc.vector.tensor_tensor(td[:, :, :m], st[:, :, 0:m], A_lo,
                                        op=AluOp.mult)
                nc.vector.tensor_tensor(st[:, :, m:2 * m], td[:, :, :m],
                                        B_lo, op=AluOp.add)
            return st

        def emit_out(c, st):
            # ---- output matmul from st (fp16); GPSIMD evacuates PSUM ----
            for s in range(4):
                col = s * SUB
                po = psum.tile([128, 2, SUB], F32, name="po", tag="ps")
                for oh in range(2):
                    for k in range(2):
                        nc.tensor.matmul(po[:, oh, :],
                                         wo_sb[k][:, oh * 128:(oh + 1) * 128],
                                         st[:, k, 1 + col:1 + col + SUB],
                                         start=(k == 0), stop=(k == 1))
                osb = osb_pool.tile([128, 2, SUB], F32, name="osb", tag="osb")
                nc.scalar.copy(osb, po)
                dst = out[:, c * L + col:c * L + col + SUB]
                nc.sync.dma_start(
                    out=dst.rearrange("(two p) n -> p two n", p=128),
                    in_=osb)

        # ---- software-pipelined emission ----
        emit_phase1(0)
        for c in range(NCHUNK):
            Aup_c, Bup_c = emit_up_top(c)
            if c + 1 < NCHUNK:
                emit_phase1(c + 1)
            st_c = emit_down(c, Aup_c, Bup_c)
            emit_out(c, st_c)

        # ---- final position T: out = otb[7]*a_T + b_T ----
        xl = singles.tile([128, 2, 1], F16, name="xl")
        nc.sync.dma_start(out=xl,
                          in_=xt[:, T:T + 1].rearrange("(k p) n -> p k n",
                                                       p=128))
        al = singles.tile([128, 2, 1], F32, name="al")
        bl = singles.tile([128, 2, 1], F32, name="bl")
        ylz = psum.tile([128, 2, SUB], F32, name="ylz", tag="ps")
        ylh = psum.tile([128, 2, SUB], F32, name="ylh", tag="ps")
        for hh in range(2):
            for k in range(2):
                nc.tensor.matmul(ylz[:, hh, 0:1],
                                 wz_sb[k][:, hh * 128:(hh + 1) * 128],
                                 xl[:, k, :], start=(k == 0), stop=(k == 1))
            for k in range(2):
                nc.tensor.matmul(ylh[:, hh, 0:1],
                                 wh_sb[k][:, hh * 128:(hh + 1) * 128],
                                 xl[:, k, :], start=(k == 0), stop=(k == 1))
            zl = singles.tile([128, 1], F32, name=f"zl{hh}")
            nc.scalar.activation(zl, ylz[:, hh, 0:1], ActFn.Sigmoid,
                                 bias=bzp_sb[hh][:, 0:1], scale=1.0)
            nc.scalar.activation(al[:, hh, :], ylz[:, hh, 0:1], ActFn.Sigmoid,
                                 bias=bzn_sb[hh][:, 0:1], scale=-1.0)
            nc.vector.scalar_tensor_tensor(bl[:, hh, :], ylh[:, hh, 0:1],
                                           bh_sb[hh][:, 0:1], zl,
                                           op0=AluOp.add, op1=AluOp.mult)
        dl = singles.tile([128, 2, 1], F32, name="dl")
        sl = singles.tile([128, 2, 1], F16, name="sl")
        nc.vector.tensor_tensor(dl, otb[:, :, 7:8], al, op=AluOp.mult)
        nc.vector.tensor_tensor(dl, dl, bl, op=AluOp.add)
        nc.scalar.copy(sl, dl)
        pol = psum.tile([128, 2, SUB], F32, name="pol", tag="ps")
        for oh in range(2):
            for k in range(2):
                nc.tensor.matmul(pol[:, oh, 0:1],
                                 wo_sb[k][:, oh * 128:(oh + 1) * 128],
                                 sl[:, k, :], start=(k == 0), stop=(k == 1))
        osl = singles.tile([128, 2, 1], F32, name="osl")
        nc.scalar.copy(osl, pol[:, :, 0:1])
        nc.sync.dma_start(
            out=out[:, T:T + 1].rearrange("(two p) n -> p two n", p=128),
            in_=osl)

    nc.compile()
    return nc


_NC_CACHE = {}


def _get_nc():
    if "nc" not in _NC_CACHE:
        _NC_CACHE["nc"] = build_nc()
    return _NC_CACHE["nc"]


def _host_indices():
    # x column source: slot r of chunk c <- x time c*L + brev(r) - 1;
    # col T = x time T-1 (for the trailing scan position)
    xsrc = np.empty(T + 1, np.int64)
    for c in range(NCHUNK):
        xsrc[c * L:(c + 1) * L] = c * L + BRT - 1
    xsrc[0] = 0  # chunk-0 slot 0 is a dummy (kernel memsets it)
    xsrc[T] = T - 1
    # hw out col -> time: col c*L + (slot-1) holds scan pos c*L+POS_OF_SLOT,
    # out time = scan pos - 1; chunk-0 pos 0 -> t=-1 (dropped)
    col_time = np.empty(T + 1, np.int64)
    for c in range(NCHUNK):
        col_time[c * L:(c + 1) * L] = c * L + POS_OF_SLOT[1:L + 1] - 1
    col_time[T] = T - 1
    time_col = np.empty(T, np.int64)
    mask = col_time >= 0
    time_col[col_time[mask]] = np.nonzero(mask)[0]
    return xsrc, time_col


_IDX_CACHE = {}


def _get_idx():
    if "i" not in _IDX_CACHE:
        _IDX_CACHE["i"] = _host_indices()
    return _IDX_CACHE["i"]


def _prepare_in_maps(xs, Wz, bz, Wh, bh, Wo, bo):
    xsrc, _ = _get_idx()
    xs = np.asarray(xs, np.float32)
    Wz = np.asarray(Wz, np.float32)
    bz = np.asarray(bz, np.float32)
    Wh = np.asarray(Wh, np.float32)
    bh = np.asarray(bh, np.float32)
    Wo = np.asarray(Wo, np.float32)

    wzt = np.ascontiguousarray(Wz.T).astype(NP16)
    wht = np.ascontiguousarray(Wh.T).astype(NP16)
    wot = np.ascontiguousarray(Wo.T).astype(NP16)
    bzp = np.ascontiguousarray(bz.reshape(H, 1))
    bzn = np.ascontiguousarray((-bz).reshape(H, 1))
    bhb = np.ascontiguousarray(bh.reshape(H, 1))

    in_maps = []
    for i in range(B):
        xti = np.ascontiguousarray(xs[i].T[:, xsrc]).astype(NP16)
        in_maps.append({
            "xt": xti, "wzt": wzt, "wht": wht, "wot": wot,
            "bzp": bzp, "bzn": bzn, "bhb": bhb,
        })
    return in_maps


def _assemble(res, bo):
    _, time_col = _get_idx()
    bo = np.asarray(bo, np.float32)
    outs = []
    for i in range(B):
        oi = np.asarray(res.results[i]["out"], np.float32)
        outs.append(oi[:, time_col].T + bo)
    return np.stack(outs, axis=0)


def run_traced(xs, Wz, bz, Wh, bh, Wo, bo, trace=True):
    in_maps = _prepare_in_maps(xs, Wz, bz, Wh, bh, Wo, bo)
    res = run_bass_kernel_spmd(_get_nc(), in_maps, core_ids=list(range(B)),
                               trace=trace)
    return _assemble(res, bo), res


def kernel(xs, Wz, bz, Wh, bh, Wo, bo):
    in_maps = _prepare_in_maps(xs, Wz, bz, Wh, bh, Wo, bo)
    res = run_bass_kernel_spmd(_get_nc(), in_maps, core_ids=list(range(B)))
    return _assemble(res, bo)
